# revision 9
# baseline (speedup 1.0000x reference)
"""Trainium2 Bass kernel for nn_ConvPool_71700184040190.

Network (per peak, 128 peaks total = B2 x 64):
  1x1 motif projection (640 -> 256), 7-layer dilated conv tower
  (k=3, dil 2..128, relu + residual crop), then two heads:
  profile conv (k=75, 256 -> 1) + softplus, and mean -> linear -> softplus.

Sharding: data-parallel over the fused batch*n_peaks axis; 16 peaks per
core on 8 cores; weights replicated.

Device layout per peak:
  - x peak tile loaded time-major, transposed on PE (identity matmul) to
    motif-major (640 = 5 x 128 partition chunks).
  - All matmuls in float32r (fp32 storage, ~tf32 matmul precision, 4x the
    fp32 matmul rate). PSUM accumulates fp32.
  - Conv-as-matmul: out[:, t] = sum_{k, cin_chunk} W[k,c].T @ h[c][:, t + k*d].
  - relu+residual fused in one DVE scalar_tensor_tensor op; last layer also
    emits the free-dim sum (accum_out) for the mean head.
  - Profile head: F = Wp.T @ h  (75 x 492), bounce through DRAM to realize
    the diagonal sum  out(t) = sum_k F[k, t+k]  as a pitch-494 strided
    reload, then a ones-vector reduction matmul + softplus.
"""

import numpy as np

B, NUM_PEAKS, PEAK_SIZE = 2, 64, 1000
MOTIF, HID, DEPTH, PROF_K = 640, 256, 7, 75
N_CORES = 8
PPC = (B * NUM_PEAKS) // N_CORES  # peaks per core = 16

_DILS = [2 ** (i + 1) for i in range(DEPTH)]
_LENS = [PEAK_SIZE]
for _d in _DILS:
    _LENS.append(_LENS[-1] - 2 * _d)
L_FINAL = _LENS[-1]              # 492
PROF_LEN = L_FINAL - PROF_K + 1  # 418

_NC_CACHE = {}


def _build(ppc):
    """Emit the per-core Bass program (same program for all cores)."""
    import concourse.bacc as bacc
    import concourse.mybir as mybir
    import concourse.tile as tile

    F32 = mybir.dt.float32
    F32R = mybir.dt.float32r
    AF = mybir.ActivationFunctionType
    OP = mybir.AluOpType

    nc = bacc.Bacc("TRN2", target_bir_lowering=False, debug=False)

    x_d = nc.dram_tensor("x", (ppc, PEAK_SIZE, MOTIF), F32, kind="ExternalInput")
    wproj_d = nc.dram_tensor("wproj", (5, 128, HID), F32, kind="ExternalInput")
    wdil_d = nc.dram_tensor("wdil", (DEPTH, 3, 2, 128, HID), F32, kind="ExternalInput")
    wprof_d = nc.dram_tensor("wprof", (2, 128, PROF_K), F32, kind="ExternalInput")
    watpm_d = nc.dram_tensor("watpm", (2, 128, 1), F32, kind="ExternalInput")
    bprof_d = nc.dram_tensor("bprof", (1, 1), F32, kind="ExternalInput")
    batpm_d = nc.dram_tensor("batpm", (1, 1), F32, kind="ExternalInput")
    ident_d = nc.dram_tensor("ident", (128, 128), F32, kind="ExternalInput")
    ones_d = nc.dram_tensor("ones75", (PROF_K, 1), F32, kind="ExternalInput")
    prof_o = nc.dram_tensor("prof", (ppc, PROF_LEN), F32, kind="ExternalOutput")
    atpm_o = nc.dram_tensor("atpm", (1, ppc), F32, kind="ExternalOutput")

    NTB = (PEAK_SIZE + 127) // 128          # 8 time blocks per peak
    LAST_TW = PEAK_SIZE - 128 * (NTB - 1)   # 104
    FRAW = PROF_K * (L_FINAL + 2)           # flat scratch, pitch 494 on reload

    with tile.TileContext(nc) as tc:
        with (
            tc.tile_pool(name="const", bufs=1) as cpool,
            tc.tile_pool(name="xa", bufs=2) as xapool,
            tc.tile_pool(name="xt", bufs=2) as xtpool,
            tc.tile_pool(name="h", bufs=5) as hpool,
            tc.tile_pool(name="small", bufs=3) as spool,
            tc.tile_pool(name="acc", bufs=1) as apool,
            tc.tile_pool(name="mpsum", bufs=2, space="PSUM") as mpsum,
            tc.tile_pool(name="tpsum", bufs=2, space="PSUM") as tpsum,
            tc.tile_pool(name="fpsum", bufs=2, space="PSUM") as fpsum,
            tc.tile_pool(name="dram", bufs=2, space="DRAM") as dpool,
        ):
            # ---- replicated constants -> SBUF ----
            wproj = cpool.tile([128, 5, HID], F32R)
            nc.sync.dma_start(wproj[:], wproj_d[:].transpose([1, 0, 2]).bitcast(F32R))
            wdil = cpool.tile([128, DEPTH, 3, 2, HID], F32R)
            nc.sync.dma_start(
                wdil[:], wdil_d[:].transpose([3, 0, 1, 2, 4]).bitcast(F32R)
            )
            wprof = cpool.tile([128, 2, PROF_K], F32R)
            nc.sync.dma_start(wprof[:], wprof_d[:].transpose([1, 0, 2]).bitcast(F32R))
            watpm = cpool.tile([128, 2, 1], F32)
            nc.sync.dma_start(watpm[:], watpm_d[:].transpose([1, 0, 2]))
            bprof = cpool.tile([1, 1], F32)
            nc.sync.dma_start(bprof[:], bprof_d[:])
            batpm = cpool.tile([1, 1], F32)
            nc.sync.dma_start(batpm[:], batpm_d[:])
            ident = cpool.tile([128, 128], F32R)
            nc.sync.dma_start(ident[:], ident_d[:].bitcast(F32R))
            ones75 = cpool.tile([PROF_K, 1], F32R)
            nc.sync.dma_start(ones75[:], ones_d[:].bitcast(F32R))

            # per-(peak, chunk) sums of final h over time, for the mean head
            atpm_sums = apool.tile([128, 2, ppc], F32)

            for p in range(ppc):
                # ---- load x peak, time-major: [t%128, t//128, motif] ----
                xa = xapool.tile([128, NTB, MOTIF], F32R, tag="xa")
                nc.sync.dma_start(
                    xa[:, 0 : NTB - 1, :],
                    x_d[p, 0 : 128 * (NTB - 1)]
                    .rearrange("(a b) c -> b a c", a=NTB - 1)
                    .bitcast(F32R),
                )
                nc.sync.dma_start(
                    xa[0:LAST_TW, NTB - 1, :],
                    x_d[p, 128 * (NTB - 1) : PEAK_SIZE].bitcast(F32R),
                )

                # ---- transpose to motif-major xT[cm, c, t] on PE ----
                xT = xtpool.tile([128, 5, PEAK_SIZE], F32R, tag="xt")
                for tb in range(NTB):
                    tw = 128 if tb < NTB - 1 else LAST_TW
                    pgroups = [(0, 4), (4, 1)]  # 4-chunk + 1-chunk psum tiles
                    for c0, cn in pgroups:
                        pt = tpsum.tile([128, 4, 128], F32R, tag="tp")
                        for c in range(cn):
                            nc.tensor.transpose(
                                pt[:, c, 0:tw],
                                xa[0:tw, tb, 128 * (c0 + c) : 128 * (c0 + c + 1)],
                                ident[0:tw, 0:tw],
                            )
                        nc.vector.tensor_copy(
                            xT[:, c0 : c0 + cn, 128 * tb : 128 * tb + tw],
                            pt[:, 0:cn, 0:tw],
                        )

                # ---- 1x1 projection: h0[ob*128+co, t] ----
                h_cur = hpool.tile([128, 2, PEAK_SIZE], F32R, tag="h")
                for ob in range(2):
                    ps = mpsum.tile([128, 2, 512], F32, tag="mp")
                    for tt in range(2):
                        t0, tw = tt * 500, 500
                        for c in range(5):
                            nc.tensor.matmul(
                                ps[:, tt, 0:tw],
                                wproj[:, c, 128 * ob : 128 * (ob + 1)],
                                xT[:, c, t0 : t0 + tw],
                                start=(c == 0),
                                stop=(c == 4),
                            )
                    nc.vector.tensor_copy(  # noqa: proj bias is zero (asserted)
                        h_cur[:, ob, :].rearrange("p (a b) -> p a b", a=2),
                        ps[:, :, 0:500],
                    )

                # ---- dilated conv tower ----
                for i in range(DEPTH):
                    d = _DILS[i]
                    l_out = _LENS[i + 1]
                    h_new = hpool.tile([128, 2, PEAK_SIZE], F32R, tag="h")
                    ntt = 2 if l_out > 512 else 1
                    tw = l_out // ntt
                    for ob in range(2):
                        ps = mpsum.tile([128, 2, 512], F32, tag="mp")
                        for tt in range(ntt):
                            t0 = tt * tw
                            for j in range(6):
                                k, c = divmod(j, 2)
                                nc.tensor.matmul(
                                    ps[:, tt, 0:tw],
                                    wdil[:, i, k, c, 128 * ob : 128 * (ob + 1)],
                                    h_cur[:, c, t0 + k * d : t0 + k * d + tw],
                                    start=(j == 0),
                                    stop=(j == 5),
                                )
                        # h_new = relu(conv) + h_cur[d : d + l_out]
                        if ntt == 2:
                            src = ps[:, :, 0:tw]
                            dst = h_new[:, ob, 0:l_out].rearrange(
                                "p (a b) -> p a b", a=2
                            )
                            res = h_cur[:, ob, d : d + l_out].rearrange(
                                "p (a b) -> p a b", a=2
                            )
                        else:
                            src = ps[:, 0, 0:l_out]
                            dst = h_new[:, ob, 0:l_out]
                            res = h_cur[:, ob, d : d + l_out]
                        nc.vector.scalar_tensor_tensor(
                            dst,
                            src,
                            0.0,
                            res,
                            OP.max,
                            OP.add,
                            accum_out=(
                                atpm_sums[:, ob, p : p + 1] if i == DEPTH - 1 else None
                            ),
                        )
                    h_cur = h_new

                # ---- profile head: F = Wp.T @ h, diagonal sum via DRAM ----
                fps = fpsum.tile([PROF_K, L_FINAL], F32, tag="fr")
                for c in range(2):
                    nc.tensor.matmul(
                        fps[:],
                        wprof[:, c, :],
                        h_cur[:, c, 0:L_FINAL],
                        start=(c == 0),
                        stop=(c == 1),
                    )
                fsb = spool.tile([PROF_K, L_FINAL], F32, tag="fsb")
                nc.scalar.copy(fsb[:], fps[:])
                fraw = dpool.tile([FRAW], F32, tag="fraw")
                # write rows at pitch L_FINAL+1 so row k reloads at k*(L_FINAL+2)
                nc.sync.dma_start(
                    fraw[0 : PROF_K * (L_FINAL + 1)].rearrange(
                        "(a b) -> a b", a=PROF_K
                    )[:, 0:L_FINAL],
                    fsb[:],
                )
                fshift = spool.tile([PROF_K, PROF_LEN], F32R, tag="fs")
                nc.sync.dma_start(
                    fshift[:],
                    fraw[:]
                    .rearrange("(a b) -> a b", a=PROF_K)[:, 0:PROF_LEN]
                    .bitcast(F32R),
                )
                rps = fpsum.tile([1, PROF_LEN], F32, tag="fr")
                nc.tensor.matmul(rps[:], ones75[:], fshift[:], start=True, stop=True)
                # softplus(v + b) = ln(1 + exp(v + b)); |v| is O(10) so the
                # naive form is exact in fp32 (no overflow / catastrophic loss)
                prof_e = spool.tile([1, PROF_LEN], F32, tag="pe")
                nc.scalar.activation(prof_e[:], rps[:], AF.Exp, bias=bprof[:])
                prof_sb = spool.tile([1, PROF_LEN], F32, tag="po")
                nc.scalar.activation(prof_sb[:], prof_e[:], AF.Ln, bias=1.0)
                nc.sync.dma_start(prof_o[p : p + 1, :], prof_sb[:])

            # ---- mean head: softplus(mean @ w_atpm + b) for all peaks ----
            # plain fp32 matmul here: tiny, and avoids an f32r rounding pass
            aps = fpsum.tile([1, ppc], F32, tag="fr")
            for c in range(2):
                nc.tensor.matmul(
                    aps[:],
                    watpm[:, c, :],
                    atpm_sums[:, c, :],
                    start=(c == 0),
                    stop=(c == 1),
                )
            atpm_e = spool.tile([1, ppc], F32, tag="pe")
            nc.scalar.activation(
                atpm_e[:], aps[:], AF.Exp, bias=batpm[:], scale=1.0 / L_FINAL
            )
            atpm_sb = spool.tile([1, ppc], F32, tag="po")
            nc.scalar.activation(atpm_sb[:], atpm_e[:], AF.Ln, bias=1.0)
            nc.sync.dma_start(atpm_o[:], atpm_sb[:])

    nc.compile()
    return nc


def _get_nc(ppc):
    if ppc not in _NC_CACHE:
        _NC_CACHE[ppc] = _build(ppc)
    return _NC_CACHE[ppc]


def _prep_inputs(x, w_proj, w_dil, w_prof, w_atpm, b_prof, b_atpm, n_cores, ppc):
    """Host-side shard + weight relayout. Returns per-core in_maps."""
    x = np.asarray(x, dtype=np.float32).reshape(B * NUM_PEAKS, PEAK_SIZE, MOTIF)
    # lhsT layouts: contraction dim on partitions
    wproj_np = np.ascontiguousarray(
        np.asarray(w_proj, np.float32)[:, :, 0].T.reshape(5, 128, HID)
    )
    wdil_np = np.ascontiguousarray(
        np.asarray(w_dil, np.float32).transpose(0, 3, 2, 1).reshape(
            DEPTH, 3, 2, 128, HID
        )
    )
    wprof_np = np.ascontiguousarray(
        np.asarray(w_prof, np.float32)[0].reshape(2, 128, PROF_K)
    )
    watpm_np = np.ascontiguousarray(
        np.asarray(w_atpm, np.float32)[0].reshape(2, 128, 1)
    )
    shared = {
        "wproj": wproj_np,
        "wdil": wdil_np,
        "wprof": wprof_np,
        "watpm": watpm_np,
        "bprof": np.asarray(b_prof, np.float32).reshape(1, 1),
        "batpm": np.asarray(b_atpm, np.float32).reshape(1, 1),
        "ident": np.eye(128, dtype=np.float32),
        "ones75": np.ones((PROF_K, 1), np.float32),
    }
    in_maps = []
    for c in range(n_cores):
        m = dict(shared)
        m["x"] = x[c * ppc : (c + 1) * ppc]
        in_maps.append(m)
    return in_maps


def kernel(x, peak_split, n_peaks, max_n_peaks,
           w_proj, b_proj, w_dil, b_dil, w_prof, b_prof, w_atpm, b_atpm):
    from concourse.bass_utils import run_bass_kernel_spmd

    assert np.all(np.asarray(b_proj) == 0) and np.all(np.asarray(b_dil) == 0), (
        "kernel specialized for zero proj/dilated biases"
    )
    nc = _get_nc(PPC)
    in_maps = _prep_inputs(
        x, w_proj, w_dil, w_prof, w_atpm, b_prof, b_atpm, N_CORES, PPC
    )
    res = run_bass_kernel_spmd(nc, in_maps, core_ids=list(range(N_CORES)))
    prof = np.concatenate([r["prof"] for r in res.results], axis=0)
    atpm = np.concatenate([r["atpm"][0] for r in res.results], axis=0)
    peak_atpm = atpm.reshape(B, NUM_PEAKS, 1).astype(np.float32)
    peak_profile = prof.reshape(B, NUM_PEAKS * PROF_LEN).astype(np.float32)
    return peak_atpm, peak_profile


# revision 12
# speedup vs baseline: 1.0982x; 1.0982x over previous
"""Trainium2 Bass kernel for nn_ConvPool_71700184040190.

Network (per peak, 128 peaks total = B2 x 64):
  1x1 motif projection (640 -> 256), 7-layer dilated conv tower
  (k=3, dil 2..128, relu + residual crop), then two heads:
  profile conv (k=75, 256 -> 1) + softplus, and mean -> linear -> softplus.

Sharding: data-parallel over the fused batch*n_peaks axis; 16 peaks per
core on 8 cores; weights replicated.

Device layout per peak:
  - x peak tile loaded time-major, transposed on PE (identity matmul) to
    motif-major (640 = 5 x 128 partition chunks).
  - All matmuls in float32r (fp32 storage, ~tf32 matmul precision, 4x the
    fp32 matmul rate). PSUM accumulates fp32.
  - Conv-as-matmul: out[:, t] = sum_{k, cin_chunk} W[k,c].T @ h[c][:, t + k*d].
  - relu+residual fused in one DVE scalar_tensor_tensor op; last layer also
    emits the free-dim sum (accum_out) for the mean head.
  - Profile head: F = Wp.T @ h  (75 x 492), bounce through DRAM to realize
    the diagonal sum  out(t) = sum_k F[k, t+k]  as a pitch-494 strided
    reload, then a ones-vector reduction matmul + softplus.
"""

import numpy as np

B, NUM_PEAKS, PEAK_SIZE = 2, 64, 1000
MOTIF, HID, DEPTH, PROF_K = 640, 256, 7, 75
N_CORES = 8
PPC = (B * NUM_PEAKS) // N_CORES  # peaks per core = 16

_DILS = [2 ** (i + 1) for i in range(DEPTH)]
_LENS = [PEAK_SIZE]
for _d in _DILS:
    _LENS.append(_LENS[-1] - 2 * _d)
L_FINAL = _LENS[-1]              # 492
PROF_LEN = L_FINAL - PROF_K + 1  # 418

_NC_CACHE = {}


def _build(ppc):
    """Emit the per-core Bass program (same program for all cores)."""
    import concourse.bacc as bacc
    import concourse.mybir as mybir
    import concourse.tile as tile

    F32 = mybir.dt.float32
    F32R = mybir.dt.float32r
    AF = mybir.ActivationFunctionType
    OP = mybir.AluOpType

    nc = bacc.Bacc("TRN2", target_bir_lowering=False, debug=False)

    x_d = nc.dram_tensor("x", (ppc, PEAK_SIZE, MOTIF), F32, kind="ExternalInput")
    wproj_d = nc.dram_tensor("wproj", (5, 128, HID), F32, kind="ExternalInput")
    wdil_d = nc.dram_tensor("wdil", (DEPTH, 3, 2, 128, HID), F32, kind="ExternalInput")
    wprof_d = nc.dram_tensor("wprof", (2, 128, PROF_K), F32, kind="ExternalInput")
    watpm_d = nc.dram_tensor("watpm", (2, 128, 1), F32, kind="ExternalInput")
    bprof_d = nc.dram_tensor("bprof", (1, 1), F32, kind="ExternalInput")
    batpm_d = nc.dram_tensor("batpm", (1, 1), F32, kind="ExternalInput")
    ident_d = nc.dram_tensor("ident", (128, 128), F32, kind="ExternalInput")
    ones_d = nc.dram_tensor("ones75", (PROF_K, 1), F32, kind="ExternalInput")
    prof_o = nc.dram_tensor("prof", (ppc, PROF_LEN), F32, kind="ExternalOutput")
    atpm_o = nc.dram_tensor("atpm", (1, ppc), F32, kind="ExternalOutput")

    NTB = (PEAK_SIZE + 127) // 128          # 8 time blocks per peak
    LAST_TW = PEAK_SIZE - 128 * (NTB - 1)   # 104
    FRAW = PROF_K * (L_FINAL + 2)           # flat scratch, pitch 494 on reload

    with tile.TileContext(nc) as tc:
        with (
            tc.tile_pool(name="const", bufs=1) as cpool,
            tc.tile_pool(name="xa", bufs=2) as xapool,
            tc.tile_pool(name="xt", bufs=2) as xtpool,
            tc.tile_pool(name="h", bufs=5) as hpool,
            tc.tile_pool(name="small", bufs=3) as spool,
            tc.tile_pool(name="acc", bufs=1) as apool,
            tc.tile_pool(name="mpsum", bufs=2, space="PSUM") as mpsum,
            tc.tile_pool(name="tpsum", bufs=2, space="PSUM") as tpsum,
            tc.tile_pool(name="fpsum", bufs=2, space="PSUM") as fpsum,
            tc.tile_pool(name="dram", bufs=2, space="DRAM") as dpool,
        ):
            # ---- replicated constants -> SBUF ----
            wproj = cpool.tile([128, 5, HID], F32R)
            nc.sync.dma_start(wproj[:], wproj_d[:].transpose([1, 0, 2]).bitcast(F32R))
            wdil = cpool.tile([128, DEPTH, 3, 2, HID], F32R)
            nc.sync.dma_start(
                wdil[:], wdil_d[:].transpose([3, 0, 1, 2, 4]).bitcast(F32R)
            )
            wprof = cpool.tile([128, 2, PROF_K], F32R)
            nc.sync.dma_start(wprof[:], wprof_d[:].transpose([1, 0, 2]).bitcast(F32R))
            watpm = cpool.tile([128, 2, 1], F32)
            nc.sync.dma_start(watpm[:], watpm_d[:].transpose([1, 0, 2]))
            bprof = cpool.tile([1, 1], F32)
            nc.sync.dma_start(bprof[:], bprof_d[:])
            batpm = cpool.tile([1, 1], F32)
            nc.sync.dma_start(batpm[:], batpm_d[:])
            ident = cpool.tile([128, 128], F32R)
            nc.sync.dma_start(ident[:], ident_d[:].bitcast(F32R))
            ones75 = cpool.tile([PROF_K, 1], F32R)
            nc.sync.dma_start(ones75[:], ones_d[:].bitcast(F32R))

            # per-(peak, chunk) sums of final h over time, for the mean head
            atpm_sums = apool.tile([128, 2, ppc], F32)

            for p in range(ppc):
                # ---- load x peak, time-major: [t%128, t//128, motif] ----
                xa = xapool.tile([128, NTB, MOTIF], F32R, tag="xa")
                nc.sync.dma_start(
                    xa[:, 0 : NTB - 1, :],
                    x_d[p, 0 : 128 * (NTB - 1)]
                    .rearrange("(a b) c -> b a c", a=NTB - 1)
                    .bitcast(F32R),
                )
                nc.sync.dma_start(
                    xa[0:LAST_TW, NTB - 1, :],
                    x_d[p, 128 * (NTB - 1) : PEAK_SIZE].bitcast(F32R),
                )

                # ---- transpose to motif-major xT[cm, c, t] on PE ----
                xT = xtpool.tile([128, 5, PEAK_SIZE], F32R, tag="xt")
                for tb in range(NTB):
                    tw = 128 if tb < NTB - 1 else LAST_TW
                    pgroups = [(0, 4), (4, 1)]  # 4-chunk + 1-chunk psum tiles
                    for c0, cn in pgroups:
                        pt = tpsum.tile([128, 4, 128], F32R, tag="tp")
                        for c in range(cn):
                            nc.tensor.transpose(
                                pt[:, c, 0:tw],
                                xa[0:tw, tb, 128 * (c0 + c) : 128 * (c0 + c + 1)],
                                ident[0:tw, 0:tw],
                            )
                        nc.vector.tensor_copy(
                            xT[:, c0 : c0 + cn, 128 * tb : 128 * tb + tw],
                            pt[:, 0:cn, 0:tw],
                        )

                # ---- 1x1 projection: h0[ob*128+co, t] ----
                h_cur = hpool.tile([128, 2, PEAK_SIZE], F32R, tag="h")
                for ob in range(2):
                    ps = mpsum.tile([128, 2, 512], F32, tag="mp")
                    for tt in range(2):
                        t0, tw = tt * 500, 500
                        for c in range(5):
                            nc.tensor.matmul(
                                ps[:, tt, 0:tw],
                                wproj[:, c, 128 * ob : 128 * (ob + 1)],
                                xT[:, c, t0 : t0 + tw],
                                start=(c == 0),
                                stop=(c == 4),
                            )
                    nc.scalar.copy(
                        h_cur[:, ob, :].rearrange("p (a b) -> p a b", a=2),
                        ps[:, :, 0:500],
                    )

                # ---- dilated conv tower ----
                for i in range(DEPTH):
                    d = _DILS[i]
                    l_out = _LENS[i + 1]
                    h_new = hpool.tile([128, 2, PEAK_SIZE], F32R, tag="h")
                    ntt = 2 if l_out > 512 else 1
                    tw = l_out // ntt
                    for ob in range(2):
                        ps = mpsum.tile([128, 2, 512], F32, tag="mp")
                        for tt in range(ntt):
                            t0 = tt * tw
                            for j in range(6):
                                k, c = divmod(j, 2)
                                nc.tensor.matmul(
                                    ps[:, tt, 0:tw],
                                    wdil[:, i, k, c, 128 * ob : 128 * (ob + 1)],
                                    h_cur[:, c, t0 + k * d : t0 + k * d + tw],
                                    start=(j == 0),
                                    stop=(j == 5),
                                )
                        # h_new = relu(conv) + h_cur[d : d + l_out]
                        if ntt == 2:
                            src = ps[:, :, 0:tw]
                            dst = h_new[:, ob, 0:l_out].rearrange(
                                "p (a b) -> p a b", a=2
                            )
                            res = h_cur[:, ob, d : d + l_out].rearrange(
                                "p (a b) -> p a b", a=2
                            )
                        else:
                            src = ps[:, 0, 0:l_out]
                            dst = h_new[:, ob, 0:l_out]
                            res = h_cur[:, ob, d : d + l_out]
                        nc.vector.scalar_tensor_tensor(
                            dst,
                            src,
                            0.0,
                            res,
                            OP.max,
                            OP.add,
                            accum_out=(
                                atpm_sums[:, ob, p : p + 1] if i == DEPTH - 1 else None
                            ),
                        )
                    h_cur = h_new

                # ---- profile head: F = Wp.T @ h, diagonal sum via DRAM ----
                fps = fpsum.tile([PROF_K, L_FINAL], F32, tag="fr")
                for c in range(2):
                    nc.tensor.matmul(
                        fps[:],
                        wprof[:, c, :],
                        h_cur[:, c, 0:L_FINAL],
                        start=(c == 0),
                        stop=(c == 1),
                    )
                fsb = spool.tile([PROF_K, L_FINAL], F32, tag="fsb")
                nc.scalar.copy(fsb[:], fps[:])
                fraw = dpool.tile([FRAW], F32, tag="fraw")
                # write rows at pitch L_FINAL+1 so row k reloads at k*(L_FINAL+2)
                nc.sync.dma_start(
                    fraw[0 : PROF_K * (L_FINAL + 1)].rearrange(
                        "(a b) -> a b", a=PROF_K
                    )[:, 0:L_FINAL],
                    fsb[:],
                )
                fshift = spool.tile([PROF_K, PROF_LEN], F32R, tag="fs")
                nc.sync.dma_start(
                    fshift[:],
                    fraw[:]
                    .rearrange("(a b) -> a b", a=PROF_K)[:, 0:PROF_LEN]
                    .bitcast(F32R),
                )
                rps = fpsum.tile([1, PROF_LEN], F32, tag="fr")
                nc.tensor.matmul(rps[:], ones75[:], fshift[:], start=True, stop=True)
                # softplus(v + b) = ln(1 + exp(v + b)); |v| is O(10) so the
                # naive form is exact in fp32 (no overflow / catastrophic loss)
                prof_e = spool.tile([1, PROF_LEN], F32, tag="pe")
                nc.scalar.activation(prof_e[:], rps[:], AF.Exp, bias=bprof[:])
                prof_sb = spool.tile([1, PROF_LEN], F32, tag="po")
                nc.scalar.activation(prof_sb[:], prof_e[:], AF.Ln, bias=1.0)
                nc.sync.dma_start(prof_o[p : p + 1, :], prof_sb[:])

            # ---- mean head: softplus(mean @ w_atpm + b) for all peaks ----
            # plain fp32 matmul here: tiny, and avoids an f32r rounding pass
            aps = fpsum.tile([1, ppc], F32, tag="fr")
            for c in range(2):
                nc.tensor.matmul(
                    aps[:],
                    watpm[:, c, :],
                    atpm_sums[:, c, :],
                    start=(c == 0),
                    stop=(c == 1),
                )
            atpm_e = spool.tile([1, ppc], F32, tag="pe")
            nc.scalar.activation(
                atpm_e[:], aps[:], AF.Exp, bias=batpm[:], scale=1.0 / L_FINAL
            )
            atpm_sb = spool.tile([1, ppc], F32, tag="po")
            nc.scalar.activation(atpm_sb[:], atpm_e[:], AF.Ln, bias=1.0)
            nc.sync.dma_start(atpm_o[:], atpm_sb[:])

    nc.compile()
    return nc


def _get_nc(ppc):
    if ppc not in _NC_CACHE:
        _NC_CACHE[ppc] = _build(ppc)
    return _NC_CACHE[ppc]


def _prep_inputs(x, w_proj, w_dil, w_prof, w_atpm, b_prof, b_atpm, n_cores, ppc):
    """Host-side shard + weight relayout. Returns per-core in_maps."""
    x = np.asarray(x, dtype=np.float32).reshape(B * NUM_PEAKS, PEAK_SIZE, MOTIF)
    # lhsT layouts: contraction dim on partitions
    wproj_np = np.ascontiguousarray(
        np.asarray(w_proj, np.float32)[:, :, 0].T.reshape(5, 128, HID)
    )
    wdil_np = np.ascontiguousarray(
        np.asarray(w_dil, np.float32).transpose(0, 3, 2, 1).reshape(
            DEPTH, 3, 2, 128, HID
        )
    )
    wprof_np = np.ascontiguousarray(
        np.asarray(w_prof, np.float32)[0].reshape(2, 128, PROF_K)
    )
    watpm_np = np.ascontiguousarray(
        np.asarray(w_atpm, np.float32)[0].reshape(2, 128, 1)
    )
    shared = {
        "wproj": wproj_np,
        "wdil": wdil_np,
        "wprof": wprof_np,
        "watpm": watpm_np,
        "bprof": np.asarray(b_prof, np.float32).reshape(1, 1),
        "batpm": np.asarray(b_atpm, np.float32).reshape(1, 1),
        "ident": np.eye(128, dtype=np.float32),
        "ones75": np.ones((PROF_K, 1), np.float32),
    }
    in_maps = []
    for c in range(n_cores):
        m = dict(shared)
        m["x"] = x[c * ppc : (c + 1) * ppc]
        in_maps.append(m)
    return in_maps


def kernel(x, peak_split, n_peaks, max_n_peaks,
           w_proj, b_proj, w_dil, b_dil, w_prof, b_prof, w_atpm, b_atpm):
    from concourse.bass_utils import run_bass_kernel_spmd

    assert np.all(np.asarray(b_proj) == 0) and np.all(np.asarray(b_dil) == 0), (
        "kernel specialized for zero proj/dilated biases"
    )
    nc = _get_nc(PPC)
    in_maps = _prep_inputs(
        x, w_proj, w_dil, w_prof, w_atpm, b_prof, b_atpm, N_CORES, PPC
    )
    res = run_bass_kernel_spmd(nc, in_maps, core_ids=list(range(N_CORES)))
    prof = np.concatenate([r["prof"] for r in res.results], axis=0)
    atpm = np.concatenate([r["atpm"][0] for r in res.results], axis=0)
    peak_atpm = atpm.reshape(B, NUM_PEAKS, 1).astype(np.float32)
    peak_profile = prof.reshape(B, NUM_PEAKS * PROF_LEN).astype(np.float32)
    return peak_atpm, peak_profile
